# revision 2
# baseline (speedup 1.0000x reference)
"""GAT conv layer (B=2, N=4096, C=256, H=4, D=64) on 8 TRN2 NeuronCores.

Execution-environment reality (measured via microbenchmarks, see mb*.py):
instructions dispatch serially within a core at ~25us (DVE) / ~35us (ACT) /
~60-100us (PE matmul) / ~32us (contiguous-1MB DMA) each, with NO overlap
across engines -- but the 8 cores DO run in parallel.  The design therefore
minimizes per-core serial instruction cost and shards across all 8 cores:
core = (batch b in 0..1) x (target-node quarter q in 0..3), i.e.
sequence-parallel over target nodes with Wh replicated (recomputed) per core.

Per core (batch b, i-span of IS=1024 target nodes):
  A: Wh-augmented projection [Wh|tgt] = x @ waug for ALL j (64 mm + 32 copies)
  B: srcB_h = broadcast(x[:, islice] @ wsrc_h) (16 mm + 4 copies)
  C: scores in [j=source (partition), i=target (free)] layout; psum holds all
     4 heads' [65, IS] accumulators (8 banks) so each mask tile is DMA'd once.
     Per (jcg, h): 4 z-STT + 1 leaky-STT + 1 Exp + 8 matmul.
  D: reciprocal of denom row, broadcast-divide via K=1 matmul, head-sum
     (0.25 head-mean baked into W on host), PE-transpose to [i, d] blocks.
"""

import numpy as np

B, N, C, H, D = 2, 4096, 256, 4, 64
NEG = 0.2
JC = N // 128        # 32 source chunks
IS = 1024            # i-span (target nodes) per core
NCORES = 8
G = 4                # j-chunks per score tile -> tiles [128, G*IS]

_cached = {}


def _build(reps=1):
    import concourse.bacc as bacc
    import concourse.tile as tile
    from concourse import mybir
    from concourse.masks import make_identity

    f32 = mybir.dt.float32
    f16 = mybir.dt.float16
    u8 = mybir.dt.uint8
    Alu = mybir.AluOpType

    nc = bacc.Bacc(None, target_bir_lowering=False, name="gat8")

    # waug column layout: for h in 0..3: [0.25*W_h (64) | w_tgt_h]
    xT = nc.dram_tensor("xT", [2, 128, N], f32, kind="ExternalInput")
    xTi = nc.dram_tensor("xTi", [2, 128, IS], f32, kind="ExternalInput")
    waug = nc.dram_tensor("waug", [2, 128, H * 65], f32, kind="ExternalInput")
    wsb = nc.dram_tensor("wsb", [H, 2, 128, 128], f32, kind="ExternalInput")
    # mprep[jcg, p, (jl, i)] = 255*mask[b, 0, q*IS+i, (jcg*4+jl)*128+p], uint8
    mprep = nc.dram_tensor("mprep", [8, 128, G * IS], u8, kind="ExternalInput")
    outd = nc.dram_tensor("out", [128, (IS // 128) * D], f32,
                          kind="ExternalOutput")

    def pipeline(tc, whaug, tgt16, srcB, ident):
        # ---------------- phase A: projection  [Wh|tgt] = x @ waug --------
        with tc.tile_pool(name="ld", bufs=1) as ld, \
             tc.tile_pool(name="psA", bufs=4, space="PSUM") as psA:
            xT_sb = ld.tile([128, 2 * N], f32)
            xTi_sb = ld.tile([128, 2 * IS], f32)
            waug_sb = ld.tile([128, 2 * H * 65], f32)
            wsb_sb = ld.tile([128, H * 2 * 128], f32)
            for cc in range(2):
                nc.sync.dma_start(xT_sb[:, cc * N:(cc + 1) * N], xT[cc])
                nc.sync.dma_start(xTi_sb[:, cc * IS:(cc + 1) * IS], xTi[cc])
                nc.sync.dma_start(
                    waug_sb[:, cc * H * 65:(cc + 1) * H * 65], waug[cc])
                for h in range(H):
                    nc.sync.dma_start(
                        wsb_sb[:, (h * 2 + cc) * 128:(h * 2 + cc + 1) * 128],
                        wsb[h, cc])

            for jc in range(JC):
                psp = psA.tile([128, H * 65], f32, tag="psp")
                for cc in range(2):
                    nc.tensor.matmul(
                        psp,
                        xT_sb[:, cc * N + jc * 128: cc * N + (jc + 1) * 128],
                        waug_sb[:, cc * H * 65:(cc + 1) * H * 65],
                        start=(cc == 0), stop=(cc == 1))
                nc.vector.tensor_copy(
                    whaug[:, jc * H * 65:(jc + 1) * H * 65], psp)
            tgt_cols = whaug.rearrange(
                "p (jch l) -> p jch l", l=65)[:, :, 64:65]
            nc.vector.tensor_copy(
                tgt16.rearrange("p (jch one) -> p jch one", one=1), tgt_cols)
            nc.vector.memset(tgt_cols, 1.0)

            # -------------- phase B: srcB_h = broadcast(x_i @ wsrc_h) -----
            with tc.tile_pool(name="psB", bufs=2, space="PSUM") as psB:
                for h in range(H):
                    pss = psB.tile([128, IS], f32, tag="pss")
                    for q in range(IS // 512):
                        for cc in range(2):
                            nc.tensor.matmul(
                                pss[:, q * 512:(q + 1) * 512],
                                wsb_sb[:, (h * 2 + cc) * 128:
                                       (h * 2 + cc + 1) * 128],
                                xTi_sb[:, cc * IS + q * 512:
                                       cc * IS + (q + 1) * 512],
                                start=(cc == 0), stop=(cc == 1),
                                skip_group_check=True)
                    nc.vector.tensor_copy(srcB[:, h * IS:(h + 1) * IS], pss)

        # ---------------- phase C: scores + attention matmul --------------
        with tc.tile_pool(name="nd", bufs=1) as ndp:
          nd = [ndp.tile([65, IS], f32, name=f"nd{h}", tag=f"nd{h}")
                for h in range(H)]
          with tc.tile_pool(name="mw", bufs=2) as mw, \
               tc.tile_pool(name="zw", bufs=2) as zw, \
               tc.tile_pool(name="lw", bufs=2) as lw, \
               tc.tile_pool(name="pw", bufs=2) as pw, \
               tc.tile_pool(name="psC", bufs=1, space="PSUM") as psC:
              acc = [psC.tile([65, IS], f32, name=f"acc{h}", tag=f"acc{h}")
                     for h in range(H)]
              for jcg in range(8):
                  m_t = mw.tile([128, G * IS], u8, tag="m")
                  nc.sync.dma_start(m_t, mprep[jcg])
                  for h in range(H):
                      z_t = zw.tile([128, G * IS], f16, tag="z")
                      for jl in range(G):
                          jc = jcg * G + jl
                          nc.vector.scalar_tensor_tensor(
                              out=z_t[:, jl * IS:(jl + 1) * IS],
                              in0=srcB[:, h * IS:(h + 1) * IS],
                              scalar=tgt16[:, jc * H + h: jc * H + h + 1],
                              in1=m_t[:, jl * IS:(jl + 1) * IS],
                              op0=Alu.add, op1=Alu.subtract)
                      l_t = lw.tile([128, G * IS], f16, tag="l")
                      nc.vector.scalar_tensor_tensor(
                          out=l_t, in0=z_t, scalar=NEG, in1=z_t,
                          op0=Alu.mult, op1=Alu.max)
                      p_t = pw.tile([128, G * IS], f16, tag="p")
                      nc.scalar.activation(
                          out=p_t, in_=l_t,
                          func=mybir.ActivationFunctionType.Exp, bias=0.0)
                      for jl in range(G):
                          jc = jcg * G + jl
                          for q in range(IS // 512):
                              nc.tensor.matmul(
                                  acc[h][:, q * 512:(q + 1) * 512],
                                  whaug[:, (jc * H + h) * 65:
                                        (jc * H + h + 1) * 65],
                                  p_t[:, jl * IS + q * 512:
                                      jl * IS + (q + 1) * 512],
                                  start=(jc == 0), stop=(jc == JC - 1),
                                  skip_group_check=True)
              for h in range(H):
                  nc.vector.tensor_copy(nd[h], acc[h])

          # ------------ phase D: normalize + head sum + transpose ---------
          with tc.tile_pool(name="psD", bufs=4, space="PSUM") as psD, \
               tc.tile_pool(name="oc", bufs=3) as oc, \
               tc.tile_pool(name="rcp", bufs=1) as rcp:
              ones65 = rcp.tile([1, 65], f16, name="ones65")
              nc.vector.memset(ones65, 1.0)
              rrows = [rcp.tile([1, IS], f16, name=f"rrow{h}", tag=f"rr{h}")
                       for h in range(H)]
              for h in range(H):
                  with nc.allow_low_precision(reason="softmax denom"):
                      nc.vector.reciprocal(rrows[h], nd[h][64:65, :])
              om = oc.tile([64, IS], f32, name="om", bufs=1)
              for iq in range(IS // 512):
                  o_prev = None
                  for h in range(H):
                      rb = psD.tile([64, 512], f32, tag="rb")
                      nc.tensor.matmul(
                          rb, ones65[0:1, 0:64],
                          rrows[h][:, iq * 512:(iq + 1) * 512],
                          start=True, stop=True, skip_group_check=True)
                      tgt_out = (om[:, iq * 512:(iq + 1) * 512]
                                 if h == H - 1 else None)
                      if h == 0:
                          o_t = oc.tile([64, 512], f32, tag="omw")
                          nc.vector.tensor_mul(
                              o_t, nd[h][0:64, iq * 512:(iq + 1) * 512], rb)
                          o_prev = o_t
                      else:
                          tmp = oc.tile([64, 512], f32, tag="omt")
                          nc.vector.tensor_mul(
                              tmp, nd[h][0:64, iq * 512:(iq + 1) * 512], rb)
                          dst = tgt_out if tgt_out is not None else oc.tile(
                              [64, 512], f32, tag="omw")
                          nc.vector.tensor_add(dst, o_prev, tmp)
                          o_prev = dst
              ob = oc.tile([128, (IS // 128) * D], f32, name="ob", bufs=1)
              for blk in range(IS // 128):
                  trp = psD.tile([128, 64], f32, tag="trp")
                  nc.tensor.transpose(
                      trp, om[:, blk * 128:(blk + 1) * 128], ident[0:64, 0:64])
                  nc.vector.tensor_copy(ob[:, blk * D:(blk + 1) * D], trp)
              nc.sync.dma_start(outd[:, :], ob)

    with tile.TileContext(nc) as tc:
        with tc.tile_pool(name="const", bufs=1) as const:
            whaug = const.tile([128, JC * H * 65], f16)
            tgt16 = const.tile([128, JC * H], f16)
            srcB = const.tile([128, H * IS], f16)
            ident = const.tile([65, 65], f32)
            make_identity(nc, ident)
            for _rep in range(reps):
                pipeline(tc, whaug, tgt16, srcB, ident)

    nc.compile()
    return nc


def _prep_inputs(x, adj_matrix_masked, W, attention):
    """Host-side shard/layout prep (slicing, transposes, weight packing)."""
    x = np.ascontiguousarray(x, dtype=np.float32)
    W = np.ascontiguousarray(W, dtype=np.float32)
    attention = np.ascontiguousarray(attention, dtype=np.float32)

    a_src = attention[:, :D, 0]          # [H, D]
    a_tgt = attention[:, D:, 0]          # [H, D]
    Wh_cols = W.reshape(C, H, D)
    w_src = np.einsum("chd,hd->ch", Wh_cols, a_src)   # [C, H]
    w_tgt = np.einsum("chd,hd->ch", Wh_cols, a_tgt)   # [C, H]

    waug = np.zeros((C, H * 65), np.float32)
    for h in range(H):
        waug[:, h * 65: h * 65 + 64] = 0.25 * Wh_cols[:, h, :]
        waug[:, h * 65 + 64] = w_tgt[:, h]
    waug = np.ascontiguousarray(waug.reshape(2, 128, H * 65))

    wsb = np.empty((H, 2, 128, 128), np.float32)
    for h in range(H):
        wsb[h] = np.repeat(
            w_src[:, h][:, None], 128, axis=1).reshape(2, 128, 128)

    in_maps = []
    for core in range(NCORES):
        b, q = divmod(core, NCORES // B)
        xTb = np.ascontiguousarray(x[b].T).reshape(2, 128, N)
        xTi = np.ascontiguousarray(
            x[b].T[:, q * IS:(q + 1) * IS]).reshape(2, 128, IS)
        # mask -> [jcg, p, (jl, i)] uint8 {0,255}; value at (jcg, p, jl, i) =
        # 255 * mask[b, 0, q*IS + i, (jcg*4 + jl)*128 + p]
        mb = adj_matrix_masked[b, 0][q * IS:(q + 1) * IS, :]   # [i-span, j]
        m = (mb.T.astype(np.uint8) * np.uint8(255))            # [j, i-span]
        m = m.reshape(8, G, 128, IS)                           # jcg, jl, p, i
        m = np.ascontiguousarray(
            m.transpose(0, 2, 1, 3).reshape(8, 128, G * IS))
        in_maps.append(dict(xT=xTb, xTi=xTi, waug=waug, wsb=wsb, mprep=m))
    return in_maps


def _run(x, adj_matrix_masked, W, attention, reps=1):
    from concourse.bass_utils import run_bass_kernel_spmd

    key = f"nc{reps}"
    if key not in _cached:
        _cached[key] = _build(reps)
    nc = _cached[key]

    in_maps = _prep_inputs(x, adj_matrix_masked, W, attention)
    res = run_bass_kernel_spmd(nc, in_maps, core_ids=list(range(NCORES)))
    out = np.empty((B, N, D), np.float32)
    for core in range(NCORES):
        b, q = divmod(core, NCORES // B)
        ob = res.results[core]["out"]                       # [128, 8*64]
        out[b, q * IS:(q + 1) * IS] = ob.reshape(
            128, IS // 128, D).transpose(1, 0, 2).reshape(IS, D)
    return out, res


def kernel(x, adj_matrix_masked, W, attention):
    out, _ = _run(x, adj_matrix_masked, W, attention)
    return out


# revision 5
# speedup vs baseline: 8.6693x; 8.6693x over previous
"""GAT conv layer (B=2, N=4096, C=256, H=4, D=64) on 8 TRN2 NeuronCores.

Execution-environment reality (measured via microbenchmarks, see mb*.py):
instructions dispatch serially within a core at ~25us (DVE) / ~35us (ACT) /
~60us (PE matmul) / ~32us (contiguous-1MB DMA) each, with NO overlap across
engines -- but the 8 cores DO run in parallel.  The design therefore
minimizes per-core serial instruction cost and shards across all 8 cores:
core = (batch b in 0..1) x (target-node quarter q in 0..3), i.e.
sequence-parallel over target nodes with Wh replicated (recomputed) per core.
Cross-engine dependencies also cost dispatch stalls, so each phase batches
same-engine instructions into long runs (per jcg: 20 DVE -> 4 ACT -> 32 PE).

Per core (batch b, i-span of IS=1024 target nodes):
  A: Wh-augmented projection [Wh|tgt] = x @ waug for ALL j (64 mm + 32 copies)
  B: srcB_h = broadcast(x[:, islice] @ wsrc_h) (16 mm + 4 copies)
  C: scores in [j=source (partition), i=target (free)] layout; psum holds all
     4 heads' [65, IS] accumulators (8 banks) so each mask tile is DMA'd once.
  D: reciprocal of denom row, broadcast-divide via K=1 matmul, head-sum
     (0.25 head-mean baked into W on host). Output stays [d, i]; host
     transposes.
"""

import numpy as np

B, N, C, H, D = 2, 4096, 256, 4, 64
NEG = 0.2
JC = N // 128        # 32 source chunks
IS = 1024            # i-span (target nodes) per core
NCORES = 8
G = 4                # j-chunks per score tile -> tiles [128, G*IS]

_cached = {}


def _build(reps=1):
    import concourse.bacc as bacc
    import concourse.tile as tile
    from concourse import mybir

    f32 = mybir.dt.float32
    f16 = mybir.dt.float16
    u8 = mybir.dt.uint8
    Alu = mybir.AluOpType

    nc = bacc.Bacc(None, target_bir_lowering=False, name="gat8")

    # waug column layout: for h in 0..3: [0.25*W_h (64) | w_tgt_h]
    xT = nc.dram_tensor("xT", [2, 128, N], f32, kind="ExternalInput")
    xTi = nc.dram_tensor("xTi", [2, 128, IS], f32, kind="ExternalInput")
    waug = nc.dram_tensor("waug", [2, 128, H * 65], f32, kind="ExternalInput")
    wsb = nc.dram_tensor("wsb", [H, 2, 128, 128], f32, kind="ExternalInput")
    # mprep[jcg, p, (jl, i)] = 255*mask[b, 0, q*IS+i, (jcg*4+jl)*128+p], uint8
    mprep = nc.dram_tensor("mprep", [8, 128, G * IS], u8, kind="ExternalInput")
    outd = nc.dram_tensor("out", [64, IS], f32, kind="ExternalOutput")

    def pipeline(tc, whaug, tgt16, srcB):
        # ---------------- phase A: projection  [Wh|tgt] = x @ waug --------
        with tc.tile_pool(name="ld", bufs=1) as ld:
            xT_sb = ld.tile([128, 2 * N], f32)
            xTi_sb = ld.tile([128, 2 * IS], f32)
            waug_sb = ld.tile([128, 2 * H * 65], f32)
            wsb_sb = ld.tile([128, H * 2 * 128], f32)
            for cc in range(2):
                nc.sync.dma_start(xT_sb[:, cc * N:(cc + 1) * N], xT[cc])
                nc.sync.dma_start(xTi_sb[:, cc * IS:(cc + 1) * IS], xTi[cc])
                nc.sync.dma_start(
                    waug_sb[:, cc * H * 65:(cc + 1) * H * 65], waug[cc])
                for h in range(H):
                    nc.sync.dma_start(
                        wsb_sb[:, (h * 2 + cc) * 128:(h * 2 + cc + 1) * 128],
                        wsb[h, cc])

            with tc.tile_pool(name="psA", bufs=1, space="PSUM") as psA:
                for jc4 in range(JC // 4):
                    psps = []
                    for jj in range(4):
                        jc = jc4 * 4 + jj
                        psp = psA.tile([128, H * 65], f32, tag=f"psp{jj}")
                        for cc in range(2):
                            nc.tensor.matmul(
                                psp,
                                xT_sb[:, cc * N + jc * 128:
                                      cc * N + (jc + 1) * 128],
                                waug_sb[:, cc * H * 65:(cc + 1) * H * 65],
                                start=(cc == 0), stop=(cc == 1))
                        psps.append(psp)
                    for jj in range(4):
                        jc = jc4 * 4 + jj
                        nc.vector.tensor_copy(
                            whaug[:, jc * H * 65:(jc + 1) * H * 65], psps[jj])
            tgt_cols = whaug.rearrange(
                "p (jch l) -> p jch l", l=65)[:, :, 64:65]
            nc.vector.tensor_copy(
                tgt16.rearrange("p (jch one) -> p jch one", one=1), tgt_cols)
            nc.vector.memset(tgt_cols, 1.0)

            # -------------- phase B: srcB_h = broadcast(x_i @ wsrc_h) -----
            with tc.tile_pool(name="psB", bufs=1, space="PSUM") as psB:
                psss = []
                for h in range(H):
                    pss = psB.tile([128, IS], f32, tag=f"pss{h}")
                    for q in range(IS // 512):
                        for cc in range(2):
                            nc.tensor.matmul(
                                pss[:, q * 512:(q + 1) * 512],
                                wsb_sb[:, (h * 2 + cc) * 128:
                                       (h * 2 + cc + 1) * 128],
                                xTi_sb[:, cc * IS + q * 512:
                                       cc * IS + (q + 1) * 512],
                                start=(cc == 0), stop=(cc == 1),
                                skip_group_check=True)
                    psss.append(pss)
                for h in range(H):
                    nc.vector.tensor_copy(
                        srcB[:, h * IS:(h + 1) * IS], psss[h])

        # ---------------- phase C: scores + attention matmul --------------
        with tc.tile_pool(name="nd", bufs=1) as ndp:
          nd = [ndp.tile([65, IS], f32, name=f"nd{h}", tag=f"nd{h}")
                for h in range(H)]
          with tc.tile_pool(name="mw", bufs=2) as mw, \
               tc.tile_pool(name="zw", bufs=1) as zw, \
               tc.tile_pool(name="lw", bufs=1) as lw, \
               tc.tile_pool(name="pw", bufs=1) as pw, \
               tc.tile_pool(name="psC", bufs=1, space="PSUM") as psC:
              acc = [psC.tile([65, IS], f32, name=f"acc{h}", tag=f"acc{h}")
                     for h in range(H)]
              for jcg in range(8):
                  m_t = mw.tile([128, G * IS], u8, tag="m")
                  nc.sync.dma_start(m_t, mprep[jcg])
                  l_ts, p_ts = [], []
                  for h in range(H):
                      z_t = zw.tile([128, G * IS], f16, tag="z")
                      for jl in range(G):
                          jc = jcg * G + jl
                          nc.vector.scalar_tensor_tensor(
                              out=z_t[:, jl * IS:(jl + 1) * IS],
                              in0=srcB[:, h * IS:(h + 1) * IS],
                              scalar=tgt16[:, jc * H + h: jc * H + h + 1],
                              in1=m_t[:, jl * IS:(jl + 1) * IS],
                              op0=Alu.add, op1=Alu.subtract)
                      l_t = lw.tile([128, G * IS], f16, tag=f"l{h}")
                      nc.vector.scalar_tensor_tensor(
                          out=l_t, in0=z_t, scalar=NEG, in1=z_t,
                          op0=Alu.mult, op1=Alu.max)
                      l_ts.append(l_t)
                  for h in range(H):
                      p_t = pw.tile([128, G * IS], f16, tag=f"p{h}")
                      nc.scalar.activation(
                          out=p_t, in_=l_ts[h],
                          func=mybir.ActivationFunctionType.Exp, bias=0.0)
                      p_ts.append(p_t)
                  for h in range(H):
                      for jl in range(G):
                          jc = jcg * G + jl
                          for q in range(IS // 512):
                              nc.tensor.matmul(
                                  acc[h][:, q * 512:(q + 1) * 512],
                                  whaug[:, (jc * H + h) * 65:
                                        (jc * H + h + 1) * 65],
                                  p_ts[h][:, jl * IS + q * 512:
                                          jl * IS + (q + 1) * 512],
                                  start=(jc == 0), stop=(jc == JC - 1),
                                  skip_group_check=True)
              for h in range(H):
                  nc.vector.tensor_copy(nd[h], acc[h])

          # ------------ phase D: normalize + head sum -------------------
          with tc.tile_pool(name="psD", bufs=1, space="PSUM") as psD, \
               tc.tile_pool(name="oc", bufs=3) as oc, \
               tc.tile_pool(name="rcp", bufs=1) as rcp:
              ones65 = rcp.tile([1, 65], f16, name="ones65")
              nc.vector.memset(ones65, 1.0)
              rrows = [rcp.tile([1, IS], f16, name=f"rrow{h}", tag=f"rr{h}")
                       for h in range(H)]
              for h in range(H):
                  with nc.allow_low_precision(reason="softmax denom"):
                      nc.vector.reciprocal(rrows[h], nd[h][64:65, :])
              rbs = {}
              for iq in range(IS // 512):
                  for h in range(H):
                      rb = psD.tile([64, 512], f32, name=f"rb{iq}{h}",
                                    tag=f"rb{iq}{h}")
                      nc.tensor.matmul(
                          rb, ones65[0:1, 0:64],
                          rrows[h][:, iq * 512:(iq + 1) * 512],
                          start=True, stop=True, skip_group_check=True)
                      rbs[(iq, h)] = rb
              om = oc.tile([64, IS], f32, name="om", bufs=1)
              for iq in range(IS // 512):
                  o_prev = None
                  for h in range(H):
                      tgt_out = (om[:, iq * 512:(iq + 1) * 512]
                                 if h == H - 1 else None)
                      tmp = oc.tile([64, 512], f32, tag=f"omt{h}")
                      nc.vector.tensor_mul(
                          tmp, nd[h][0:64, iq * 512:(iq + 1) * 512],
                          rbs[(iq, h)])
                      if h == 0:
                          o_prev = tmp
                      else:
                          dst = tgt_out if tgt_out is not None else oc.tile(
                              [64, 512], f32, tag=f"omw{h}")
                          nc.vector.tensor_add(dst, o_prev, tmp)
                          o_prev = dst
              nc.sync.dma_start(outd[:, :], om)

    with tile.TileContext(nc) as tc:
        with tc.tile_pool(name="const", bufs=1) as const:
            whaug = const.tile([128, JC * H * 65], f16)
            tgt16 = const.tile([128, JC * H], f16)
            srcB = const.tile([128, H * IS], f16)
            for _rep in range(reps):
                pipeline(tc, whaug, tgt16, srcB)

    nc.compile()
    return nc


def _prep_inputs(x, adj_matrix_masked, W, attention):
    """Host-side shard/layout prep (slicing, transposes, weight packing)."""
    x = np.ascontiguousarray(x, dtype=np.float32)
    W = np.ascontiguousarray(W, dtype=np.float32)
    attention = np.ascontiguousarray(attention, dtype=np.float32)

    a_src = attention[:, :D, 0]          # [H, D]
    a_tgt = attention[:, D:, 0]          # [H, D]
    Wh_cols = W.reshape(C, H, D)
    w_src = np.einsum("chd,hd->ch", Wh_cols, a_src)   # [C, H]
    w_tgt = np.einsum("chd,hd->ch", Wh_cols, a_tgt)   # [C, H]

    waug = np.zeros((C, H * 65), np.float32)
    for h in range(H):
        waug[:, h * 65: h * 65 + 64] = 0.25 * Wh_cols[:, h, :]
        waug[:, h * 65 + 64] = w_tgt[:, h]
    waug = np.ascontiguousarray(waug.reshape(2, 128, H * 65))

    wsb = np.empty((H, 2, 128, 128), np.float32)
    for h in range(H):
        wsb[h] = np.repeat(
            w_src[:, h][:, None], 128, axis=1).reshape(2, 128, 128)

    in_maps = []
    for core in range(NCORES):
        b, q = divmod(core, NCORES // B)
        xTb = np.ascontiguousarray(x[b].T).reshape(2, 128, N)
        xTi = np.ascontiguousarray(
            x[b].T[:, q * IS:(q + 1) * IS]).reshape(2, 128, IS)
        # mask -> [jcg, p, (jl, i)] uint8 {0,255}; value at (jcg, p, jl, i) =
        # 255 * mask[b, 0, q*IS + i, (jcg*4 + jl)*128 + p]
        mb = adj_matrix_masked[b, 0][q * IS:(q + 1) * IS, :]   # [i-span, j]
        m = (mb.T.astype(np.uint8) * np.uint8(255))            # [j, i-span]
        m = m.reshape(8, G, 128, IS)                           # jcg, jl, p, i
        m = np.ascontiguousarray(
            m.transpose(0, 2, 1, 3).reshape(8, 128, G * IS))
        in_maps.append(dict(xT=xTb, xTi=xTi, waug=waug, wsb=wsb, mprep=m))
    return in_maps


def _run(x, adj_matrix_masked, W, attention, reps=1):
    from concourse.bass_utils import run_bass_kernel_spmd

    key = f"nc{reps}"
    if key not in _cached:
        _cached[key] = _build(reps)
    nc = _cached[key]

    in_maps = _prep_inputs(x, adj_matrix_masked, W, attention)
    res = run_bass_kernel_spmd(nc, in_maps, core_ids=list(range(NCORES)))
    out = np.empty((B, N, D), np.float32)
    for core in range(NCORES):
        b, q = divmod(core, NCORES // B)
        om = res.results[core]["out"]                       # [64, IS]
        out[b, q * IS:(q + 1) * IS] = om.T
    return out, res


def kernel(x, adj_matrix_masked, W, attention):
    out, _ = _run(x, adj_matrix_masked, W, attention)
    return out


# revision 7
# speedup vs baseline: 16.7275x; 1.9295x over previous
"""GAT conv layer (B=2, N=4096, C=256, H=4, D=64) on 8 TRN2 NeuronCores.

Execution-environment reality (measured via microbenchmarks, see mb*.py):
per-engine sequencers dispatch at ~25us (DVE) / ~35us (ACT) / ~60us (PE
matmul+ldweights) per instruction; engine streams can overlap when the
dependency structure allows, and the 8 cores run in parallel.  The PE stream
(~540 instructions/core) is the critical path, so the design (a) shards
across all 8 cores: core = (batch b) x (target-node quarter q), i.e.
sequence-parallel over target nodes with Wh replicated (recomputed) per
core, and (b) interleaves DVE/ACT score work with PE aggregation matmuls so
the score pipeline hides under PE dispatch.

Per core (batch b, i-span of IS=1024 target nodes):
  A: Wh-augmented projection [Wh|tgt] = x @ waug for ALL j (64 mm + 32 copies)
  B: srcB_h = broadcast(x[:, islice] @ wsrc_h) (16 mm + 4 copies)
  C: scores in [j=source (partition), i=target (free)] layout; psum holds all
     4 heads' [65, IS] accumulators (8 banks) so each mask tile is DMA'd once.
     Per (jcg, h): 4 z-STT + 1 leaky-STT + 1 Exp + 8 matmul, interleaved.
  D: reciprocal of denom row, broadcast-divide via K=1 matmul, head-sum
     (0.25 head-mean baked into W on host). Output stays [d, i]; host
     transposes.
"""

import numpy as np

B, N, C, H, D = 2, 4096, 256, 4, 64
NEG = 0.2
JC = N // 128        # 32 source chunks
IS = 1024            # i-span (target nodes) per core
NCORES = 8
G = 4                # j-chunks per score tile -> tiles [128, G*IS]

_cached = {}


def _build(reps=1, ablate=None):
    import concourse.bacc as bacc
    import concourse.tile as tile
    from concourse import mybir

    f32 = mybir.dt.float32
    f16 = mybir.dt.float16
    u8 = mybir.dt.uint8
    Alu = mybir.AluOpType

    nc = bacc.Bacc(None, target_bir_lowering=False, name="gat8")

    # packed inputs (single contiguous DMA each):
    # xTp[p, (cc, n)]: x[b].T row-split into two 128-partition halves
    xTp = nc.dram_tensor("xTp", [128, 2 * N], f32, kind="ExternalInput")
    xTip = nc.dram_tensor("xTip", [128, 2 * IS], f32, kind="ExternalInput")
    # waugp[p, (cc, h*65)]: for h in 0..3: [0.25*W_h (64) | w_tgt_h]
    waugp = nc.dram_tensor("waugp", [128, 2 * H * 65], f32,
                           kind="ExternalInput")
    # wsbp[p, (h, cc, 128)]: w_src columns broadcast
    wsbp = nc.dram_tensor("wsbp", [128, H * 2 * 128], f32,
                          kind="ExternalInput")
    # mprep[jcg, p, (jl, i)] = 255*mask[b, 0, q*IS+i, (jcg*4+jl)*128+p], uint8
    mprep = nc.dram_tensor("mprep", [8, 128, G * IS], u8, kind="ExternalInput")
    outd = nc.dram_tensor("out", [64, IS], f32, kind="ExternalOutput")

    def pipeline(tc, whaug, tgt16, srcB, ablate=None):
        # ---------------- phase A: projection  [Wh|tgt] = x @ waug --------
        with tc.tile_pool(name="ld", bufs=1) as ld:
            xT_sb = ld.tile([128, 2 * N], f32)
            xTi_sb = ld.tile([128, 2 * IS], f32)
            waug_sb = ld.tile([128, 2 * H * 65], f32)
            wsb_sb = ld.tile([128, H * 2 * 128], f32)
            nc.sync.dma_start(xT_sb, xTp[:, :])
            nc.sync.dma_start(xTi_sb, xTip[:, :])
            nc.sync.dma_start(waug_sb, waugp[:, :])
            nc.sync.dma_start(wsb_sb, wsbp[:, :])

            with tc.tile_pool(name="psA", bufs=4, space="PSUM") as psA:
                for jc in range(JC):
                    psp = psA.tile([128, H * 65], f32, tag="psp")
                    for cc in range(2):
                        nc.tensor.matmul(
                            psp,
                            xT_sb[:, cc * N + jc * 128:
                                  cc * N + (jc + 1) * 128],
                            waug_sb[:, cc * H * 65:(cc + 1) * H * 65],
                            start=(cc == 0), stop=(cc == 1))
                    nc.vector.tensor_copy(
                        whaug[:, jc * H * 65:(jc + 1) * H * 65], psp)
            tgt_cols = whaug.rearrange(
                "p (jch l) -> p jch l", l=65)[:, :, 64:65]
            nc.vector.tensor_copy(
                tgt16.rearrange("p (jch one) -> p jch one", one=1), tgt_cols)
            nc.vector.memset(tgt_cols, 1.0)

            # -------------- phase B: srcB_h = broadcast(x_i @ wsrc_h) -----
            with tc.tile_pool(name="psB", bufs=2, space="PSUM") as psB:
                for h in range(H):
                    pss = psB.tile([128, IS], f32, tag="pss")
                    for cc in range(2):
                        for q in range(IS // 512):
                            nc.tensor.matmul(
                                pss[:, q * 512:(q + 1) * 512],
                                wsb_sb[:, (h * 2 + cc) * 128:
                                       (h * 2 + cc + 1) * 128],
                                xTi_sb[:, cc * IS + q * 512:
                                       cc * IS + (q + 1) * 512],
                                start=(cc == 0), stop=(cc == 1),
                                skip_group_check=True)
                    nc.vector.tensor_copy(srcB[:, h * IS:(h + 1) * IS], pss)

        # ---------------- phase C: scores + attention matmul --------------
        with tc.tile_pool(name="nd", bufs=1) as ndp:
          nd = [ndp.tile([65, IS], f32, name=f"nd{h}", tag=f"nd{h}")
                for h in range(H)]
          with tc.tile_pool(name="mw", bufs=3) as mw, \
               tc.tile_pool(name="zw", bufs=2) as zw, \
               tc.tile_pool(name="lw", bufs=2) as lw, \
               tc.tile_pool(name="pw", bufs=3) as pw, \
               tc.tile_pool(name="psC", bufs=1, space="PSUM") as psC:
              acc = [psC.tile([65, IS], f32, name=f"acc{h}", tag=f"acc{h}")
                     for h in range(H)]
              for jcg in range(8):
                  m_t = mw.tile([128, G * IS], u8, tag="m")
                  nc.sync.dma_start(m_t, mprep[jcg])
                  for h in range(H):
                      p_t = pw.tile([128, G * IS], f16, tag="p")
                      if ablate == "noscore":
                          nc.vector.memset(p_t, 0.01)
                      else:
                          z_t = zw.tile([128, G * IS], f16, tag="z")
                          for jl in range(G):
                              jc = jcg * G + jl
                              nc.vector.scalar_tensor_tensor(
                                  out=z_t[:, jl * IS:(jl + 1) * IS],
                                  in0=srcB[:, h * IS:(h + 1) * IS],
                                  scalar=tgt16[:, jc * H + h: jc * H + h + 1],
                                  in1=m_t[:, jl * IS:(jl + 1) * IS],
                                  op0=Alu.add, op1=Alu.subtract)
                          l_t = lw.tile([128, G * IS], f16, tag="l")
                          nc.vector.scalar_tensor_tensor(
                              out=l_t, in0=z_t, scalar=NEG, in1=z_t,
                              op0=Alu.mult, op1=Alu.max)
                          nc.scalar.activation(
                              out=p_t, in_=l_t,
                              func=mybir.ActivationFunctionType.Exp, bias=0.0)
                      if ablate == "noCmm":
                          continue
                      for jl in range(G):
                          jc = jcg * G + jl
                          for q in range(IS // 512):
                              nc.tensor.matmul(
                                  acc[h][:, q * 512:(q + 1) * 512],
                                  whaug[:, (jc * H + h) * 65:
                                        (jc * H + h + 1) * 65],
                                  p_t[:, jl * IS + q * 512:
                                      jl * IS + (q + 1) * 512],
                                  start=(jc == 0), stop=(jc == JC - 1),
                                  skip_group_check=True)
              for h in range(H):
                  if ablate == "noCmm":
                      nc.vector.memset(nd[h], 1.0)
                  else:
                      nc.vector.tensor_copy(nd[h], acc[h])

          # ------------ phase D: normalize + head sum -------------------
          with tc.tile_pool(name="psD", bufs=4, space="PSUM") as psD, \
               tc.tile_pool(name="oc", bufs=3) as oc, \
               tc.tile_pool(name="rcp", bufs=1) as rcp:
              ones65 = rcp.tile([1, 65], f16, name="ones65")
              nc.vector.memset(ones65, 1.0)
              rrows = [rcp.tile([1, IS], f16, name=f"rrow{h}", tag=f"rr{h}")
                       for h in range(H)]
              for h in range(H):
                  with nc.allow_low_precision(reason="softmax denom"):
                      nc.vector.reciprocal(rrows[h], nd[h][64:65, :])
              om = oc.tile([64, IS], f32, name="om", bufs=1)
              for iq in range(IS // 512):
                  o_prev = None
                  for h in range(H):
                      rb = psD.tile([64, 512], f32, tag="rb")
                      nc.tensor.matmul(
                          rb, ones65[0:1, 0:64],
                          rrows[h][:, iq * 512:(iq + 1) * 512],
                          start=True, stop=True, skip_group_check=True)
                      tgt_out = (om[:, iq * 512:(iq + 1) * 512]
                                 if h == H - 1 else None)
                      tmp = oc.tile([64, 512], f32, tag=f"omt{h}")
                      nc.vector.tensor_mul(
                          tmp, nd[h][0:64, iq * 512:(iq + 1) * 512], rb)
                      if h == 0:
                          o_prev = tmp
                      else:
                          dst = tgt_out if tgt_out is not None else oc.tile(
                              [64, 512], f32, tag=f"omw{h}")
                          nc.vector.tensor_add(dst, o_prev, tmp)
                          o_prev = dst
              nc.sync.dma_start(outd[:, :], om)

    with tile.TileContext(nc) as tc:
        with tc.tile_pool(name="const", bufs=1) as const:
            whaug = const.tile([128, JC * H * 65], f16)
            tgt16 = const.tile([128, JC * H], f16)
            srcB = const.tile([128, H * IS], f16)
            for _rep in range(reps):
                pipeline(tc, whaug, tgt16, srcB, ablate)

    nc.compile()
    return nc


def _prep_inputs(x, adj_matrix_masked, W, attention):
    """Host-side shard/layout prep (slicing, transposes, weight packing)."""
    x = np.ascontiguousarray(x, dtype=np.float32)
    W = np.ascontiguousarray(W, dtype=np.float32)
    attention = np.ascontiguousarray(attention, dtype=np.float32)

    a_src = attention[:, :D, 0]          # [H, D]
    a_tgt = attention[:, D:, 0]          # [H, D]
    Wh_cols = W.reshape(C, H, D)
    w_src = np.einsum("chd,hd->ch", Wh_cols, a_src)   # [C, H]
    w_tgt = np.einsum("chd,hd->ch", Wh_cols, a_tgt)   # [C, H]

    waug = np.zeros((C, H * 65), np.float32)
    for h in range(H):
        waug[:, h * 65: h * 65 + 64] = 0.25 * Wh_cols[:, h, :]
        waug[:, h * 65 + 64] = w_tgt[:, h]
    # pack [128, (cc, H*65)]
    waugp = np.ascontiguousarray(
        np.concatenate([waug[:128], waug[128:]], axis=1))

    # wsbp[p, (h, cc, 128)]
    wsbp = np.empty((128, H * 2 * 128), np.float32)
    for h in range(H):
        for cc in range(2):
            wsbp[:, (h * 2 + cc) * 128:(h * 2 + cc + 1) * 128] = np.repeat(
                w_src[cc * 128:(cc + 1) * 128, h][:, None], 128, axis=1)
    wsbp = np.ascontiguousarray(wsbp)

    in_maps = []
    for core in range(NCORES):
        b, q = divmod(core, NCORES // B)
        xb = x[b].T                                    # [C, N]
        xTp = np.ascontiguousarray(
            np.concatenate([xb[:128], xb[128:]], axis=1))       # [128, 2N]
        xi = xb[:, q * IS:(q + 1) * IS]
        xTip = np.ascontiguousarray(
            np.concatenate([xi[:128], xi[128:]], axis=1))       # [128, 2*IS]
        # mask -> [jcg, p, (jl, i)] uint8 {0,255}; value at (jcg, p, jl, i) =
        # 255 * mask[b, 0, q*IS + i, (jcg*4 + jl)*128 + p]
        mb = adj_matrix_masked[b, 0][q * IS:(q + 1) * IS, :]   # [i-span, j]
        m = (mb.T.astype(np.uint8) * np.uint8(255))            # [j, i-span]
        m = m.reshape(8, G, 128, IS)                           # jcg, jl, p, i
        m = np.ascontiguousarray(
            m.transpose(0, 2, 1, 3).reshape(8, 128, G * IS))
        in_maps.append(dict(xTp=xTp, xTip=xTip, waugp=waugp, wsbp=wsbp,
                            mprep=m))
    return in_maps


def _run(x, adj_matrix_masked, W, attention, reps=1):
    from concourse.bass_utils import run_bass_kernel_spmd

    key = f"nc{reps}"  # ablate variants not cached
    if key not in _cached:
        _cached[key] = _build(reps)
    nc = _cached[key]

    in_maps = _prep_inputs(x, adj_matrix_masked, W, attention)
    res = run_bass_kernel_spmd(nc, in_maps, core_ids=list(range(NCORES)))
    out = np.empty((B, N, D), np.float32)
    for core in range(NCORES):
        b, q = divmod(core, NCORES // B)
        om = res.results[core]["out"]                       # [64, IS]
        out[b, q * IS:(q + 1) * IS] = om.T
    return out, res


def kernel(x, adj_matrix_masked, W, attention):
    out, _ = _run(x, adj_matrix_masked, W, attention)
    return out
